# revision 12
# baseline (speedup 1.0000x reference)
"""Expert-parallel MoE GEGLU MLP (RMSNorm -> c_fc -> GEGLU -> c_proj) on 8
Trainium2 NeuronCores.

Sharding: expert-parallel. Core e computes the full MLP for expert e's tokens
(x[:, e] -> [8192, 768]); no collectives. gamma*sqrt(D) is folded into c_fc
and mult_bias into c_proj on the host, so the device kernel computes:

    h   = x / ||x||_2            (per token, fp32 accumulate)
    u   = h @ W1                 (bf16 x bf16 -> fp32 PSUM)
    g   = gelu(u_gate) * u_val   (exact erf gelu on ACT)
    out = g @ W2                 (bf16 x bf16 -> fp32 PSUM)

Layout: tokens stream in super-blocks of 1024. x is loaded twice: once
token-major (for the squared-sum only) and once d-major via the DMA xbar
transpose straight from DRAM. The per-token rsqrt scale is computed
token-major (cheap DVE Newton), moved to a row with one tiny PE transpose,
broadcast across partitions with K=8 bf16 hi/lo matmul pairs that accumulate
in PSUM (full fp32-grade precision at 1 cycle/row; the old fp32 selector
matmuls ran at 4 cycles/row with double LDWEIGHTS), and applied in place to
the transposed activations with DVE reads straight out of PSUM. GEMM1 runs
with hidden on PSUM partitions and 1024-token moving operands; GEMM2 uses the
GEGLU output chunks as the stationary operand so its PSUM output is already
token-major - no output transposes at all.

DMA queues: the latency-critical x stream (xb + xT) owns the sync queue; the
19MB weight bulk streams on the scalar queue behind only the first w1 column
pair; outputs drain on gpsimd. The first super-block's xb lands in two halves
so the ACT square chain starts ~4us earlier, and a short burst of dummy
matmuls warms the PE p-state while the first DMAs are in flight.
"""

from contextlib import ExitStack

import ml_dtypes
import numpy as np

import concourse.bass as bass
import concourse.mybir as mybir
import concourse.tile as tile
from concourse import bacc
from concourse.bass_utils import run_bass_kernel_spmd
from concourse.masks import make_identity

# Problem dims (fixed by the nn_MLP_90795608637901 spec).
B, E, CAP, D = 8, 8, 1024, 768
H = 2048
H2 = 2 * H
T = B * CAP          # tokens per expert (per core) = 8192
SB = 1024            # tokens per super-block
NSB = T // SB        # 8
S = SB // 128        # 8 partition sub-tiles per super-block
KC1 = D // 128       # 6 contraction chunks for GEMM1
MC = H // 128        # 16 value/gate chunk pairs
KC2 = H // 128       # 16 contraction chunks for GEMM2

BF = mybir.dt.bfloat16
F32 = mybir.dt.float32
I32 = mybir.dt.int32
ALU = mybir.AluOpType


def build_kernel(nsb: int = NSB) -> bass.Bass:
    nc = bacc.Bacc("TRN2", target_bir_lowering=False, debug=False)

    t = nsb * SB
    x = nc.declare_dram_parameter("x", [t, D], BF, isOutput=False)
    xT = nc.declare_dram_parameter("xT", [D, t], BF, isOutput=False)
    w1 = nc.declare_dram_parameter("w1", [D, H2], BF, isOutput=False)
    w2 = nc.declare_dram_parameter("w2", [H, D], BF, isOutput=False)
    sel = nc.declare_dram_parameter("sel", [S, SB], BF, isOutput=False)
    out = nc.declare_dram_parameter("out", [t, D], BF, isOutput=True)

    with tile.TileContext(nc) as tc, ExitStack() as ctx:
        weights = ctx.enter_context(tc.tile_pool(name="weights", bufs=1))
        io_in = ctx.enter_context(tc.tile_pool(name="io_in", bufs=2))
        work = ctx.enter_context(tc.tile_pool(name="work", bufs=2))
        gpool = ctx.enter_context(tc.tile_pool(name="gpool", bufs=1))
        small = ctx.enter_context(tc.tile_pool(name="small", bufs=2))
        agp = ctx.enter_context(tc.tile_pool(name="agp", bufs=3))
        obp = ctx.enter_context(tc.tile_pool(name="obp", bufs=3))
        psum_mm = ctx.enter_context(tc.tile_pool(name="psum_mm", bufs=5, space="PSUM"))
        psum_sc = ctx.enter_context(tc.tile_pool(name="psum_sc", bufs=1, space="PSUM"))
        psum_yt = ctx.enter_context(tc.tile_pool(name="psum_yt", bufs=1, space="PSUM"))

        # x DMAs for a super-block, all on the sync queue (the x stream is
        # latency-critical; the weight bulk rides the scalar queue).
        x_tiles = {}

        def issue_x(sb):
            xb = io_in.tile([128, S, D], BF, name="xb", tag="xb")
            xv = x[sb * SB:(sb + 1) * SB].rearrange("(s p) d -> p s d", p=128)
            nc.sync.dma_start(out=xb, in_=xv)
            xt = work.tile([128, KC1, SB], BF, name="xt", tag="xt")
            for k in range(KC1):
                nc.sync.dma_start(
                    out=xt[:, k, :],
                    in_=xT[k * 128:(k + 1) * 128, sb * SB:(sb + 1) * SB],
                )
            x_tiles[sb] = (xb, xt)

        # --- tiny constants (gpsimd/vector, ~0) ---
        ident = weights.tile([128, 128], F32)
        make_identity(nc, ident)
        # sel[s, s*128+q] = 1: selector for the partition-broadcast matmul
        # (bf16: 0/1 are exact).
        sels = weights.tile([S, SB], BF)
        bias0 = weights.tile([128, 1], F32)
        nc.vector.memset(bias0, 0.0)
        warm = weights.tile([128, 128], BF)
        nc.gpsimd.memset(warm, 0.0)

        # --- startup-ordered DMA head ---
        # sync queue: xb0 in two halves (ACT squares start after half 1),
        # then xt0. scalar queue: first w1 column pair, then the bulk.
        xb0 = io_in.tile([128, S, D], BF, name="xb", tag="xb")
        for h in range(2):
            sl = slice(h * (SB // 2), (h + 1) * (SB // 2))
            nc.sync.dma_start(
                out=xb0[:, h * (S // 2):(h + 1) * (S // 2), :],
                in_=x[sl].rearrange("(s p) d -> p s d", p=128),
            )
        xt0 = work.tile([128, KC1, SB], BF, name="xt", tag="xt")
        for k in range(KC1):
            nc.sync.dma_start(out=xt0[:, k, :], in_=xT[k * 128:(k + 1) * 128, 0:SB])
        x_tiles[0] = (xb0, xt0)
        nc.sync.dma_start(out=sels, in_=sel[:, :])

        # Warm the PE p-state while the first DMAs stream: ~24 dummy
        # 128-row matmuls on a zeroed tile (no data deps, scheduled at t~8us).
        pwarm = psum_yt.tile([128, 128], F32, name="pwarm", tag="yt", space="PSUM")
        for _ in range(24):
            nc.tensor.matmul(pwarm, lhsT=warm, rhs=warm, start=True, stop=True)

        # Weight bulk: the DMA fabric round-robins all in-flight transfers,
        # so arrival order == issue order only if the stream is serialized.
        # Everything rides the sync queue in need-order behind sb0's x, as
        # few large rearranged DMAs (~0.6us issue each): one per 512-column
        # w1 value/gate block, one for all of w2.
        w1s = weights.tile([128, KC1, H2], BF)
        for nb in range(4):
            for base in (0, H):
                c0, c1 = base + nb * 512, base + (nb + 1) * 512
                nc.sync.dma_start(
                    out=w1s[:, :, c0:c1],
                    in_=w1[:, c0:c1].rearrange("(k p) c -> p k c", p=128),
                )
        w2s = weights.tile([128, KC2, D], BF)
        nc.sync.dma_start(
            out=w2s, in_=w2[:, :].rearrange("(k p) d -> p k d", p=128),
        )

        normed = {}

        def norm_pipeline(sb):
            xb, xt = x_tiles.pop(sb)
            # --- RMSNorm scale, token-major: ss on ACT, rsqrt on DVE ---
            # For the first super-block the chain is the startup critical
            # path, so half the squared-sums run on DVE concurrently.
            ssb = small.tile([128, S], F32, name="ssb")
            sq = small.tile([128, D], BF, name="sq")
            for s in range(S):
                nc.scalar.activation(
                    sq, xb[:, s], mybir.ActivationFunctionType.Square,
                    bias=bias0, accum_out=ssb[:, s:s + 1],
                )
            yb = small.tile([128, S], F32, name="yb")
            tb = small.tile([128, S], F32, name="tb")
            # rsqrt seed via the int bit trick: 0x5f3759df - (i >> 1)
            # (written as (i>>1 xor -1) + 0x5f3759df + 1), then 3 Newton steps.
            nc.vector.tensor_scalar(
                out=yb.bitcast(I32), in0=ssb.bitcast(I32),
                scalar1=1, scalar2=-1,
                op0=ALU.logical_shift_right, op1=ALU.bitwise_xor,
            )
            nc.vector.tensor_scalar(
                out=yb.bitcast(I32), in0=yb.bitcast(I32),
                scalar1=0x5F375A60, scalar2=None, op0=ALU.add,
            )
            # Two Newton steps suffice: seed err <= 3.4% -> 1.7e-3 -> 4.4e-6.
            for _ in range(2):
                nc.vector.tensor_mul(tb, yb, yb)
                nc.vector.tensor_mul(tb, tb, ssb)
                nc.vector.tensor_scalar(
                    out=tb, in0=tb, scalar1=-0.5, scalar2=1.5,
                    op0=ALU.mult, op1=ALU.add,
                )
                nc.vector.tensor_mul(yb, yb, tb)

            # --- broadcast scale across partitions: yb[p,s] -> psc[:,s*128+p]
            # One tiny fp32 transpose, then bf16 hi/lo selector matmul pairs
            # accumulating in PSUM: exact to ~16 mantissa bits, 1 cycle/row.
            yt = psum_yt.tile([S, 128], F32, name="yt", tag="yt", space="PSUM")
            nc.tensor.transpose(yt, yb, ident)
            yrow = small.tile([S, 128], F32, name="yrow")
            nc.vector.tensor_copy(yrow, yt)
            yhi = small.tile([S, 128], BF, name="yhi")
            nc.vector.tensor_copy(yhi, yrow)
            ylo = small.tile([S, 128], BF, name="ylo")
            nc.vector.tensor_sub(ylo, yrow, yhi)
            psc = psum_sc.tile([128, SB], F32, name="psc", tag="sc", space="PSUM")
            for s in range(S):
                cols = slice(s * 128, (s + 1) * 128)
                nc.tensor.matmul(psc[:, cols], lhsT=sels[:, cols], rhs=yhi,
                                 start=True, stop=False)
                nc.tensor.matmul(psc[:, cols], lhsT=sels[:, cols], rhs=ylo,
                                 start=False, stop=True)

            # --- normalize in place in the transposed domain (DVE reads the
            # scale straight out of PSUM). Half-column passes so GEMM1's
            # first 512-token chains start after 6 muls, not 12. ---
            for hc in range(2):
                cols = slice(hc * 512, (hc + 1) * 512)
                for k in range(KC1):
                    nc.vector.tensor_mul(xt[:, k, cols], xt[:, k, cols],
                                         psc[:, cols])
            normed[sb] = xt

        norm_pipeline(0)
        for sb in range(nsb):
            if sb + 1 < nsb:
                issue_x(sb + 1)
            xt = normed.pop(sb)

            # --- GEMM1 + GEGLU, one value/gate chunk pair at a time.
            # A matmul's fp32 PSUM output cannot cross a 2KB bank, so the
            # 1024-token super-block runs as two 512-column halves. ---
            gbuf = gpool.tile([128, KC2, SB], BF, name="gbuf")
            for m in range(MC):
                for h2 in range(2):
                    cols = slice(h2 * 512, (h2 + 1) * 512)
                    pv = psum_mm.tile([128, 512], F32, name="pv", tag="mm",
                                      space="PSUM")
                    pg = psum_mm.tile([128, 512], F32, name="pg", tag="mm",
                                      space="PSUM")
                    for k in range(KC1):
                        nc.tensor.matmul(
                            pv, lhsT=w1s[:, k, m * 128:(m + 1) * 128],
                            rhs=xt[:, k, cols],
                            start=(k == 0), stop=(k == KC1 - 1),
                        )
                    for k in range(KC1):
                        nc.tensor.matmul(
                            pg, lhsT=w1s[:, k, H + m * 128:H + (m + 1) * 128],
                            rhs=xt[:, k, cols],
                            start=(k == 0), stop=(k == KC1 - 1),
                        )
                    ag = agp.tile([128, 512], F32, name="ag")
                    nc.scalar.activation(
                        ag, pg, mybir.ActivationFunctionType.Gelu, bias=bias0,
                    )
                    nc.vector.tensor_mul(gbuf[:, m, cols], pv, ag)

            if sb + 1 < nsb:
                norm_pipeline(sb + 1)

            # --- GEMM2 with gbuf chunks stationary: PSUM comes out
            # token-major, so results DMA straight out after one copy.
            # d=768 output splits into 512+256 PSUM chains (bank rule). ---
            for mt in range(S):
                ob = obp.tile([128, D], BF, name="ob")
                for d0, d1 in ((0, 512), (512, 768)):
                    po = psum_mm.tile([128, d1 - d0], F32, name="po", tag="mm",
                                      space="PSUM")
                    for k2 in range(KC2):
                        nc.tensor.matmul(
                            po, lhsT=gbuf[:, k2, mt * 128:(mt + 1) * 128],
                            rhs=w2s[:, k2, d0:d1],
                            start=(k2 == 0), stop=(k2 == KC2 - 1),
                        )
                    nc.vector.tensor_copy(ob[:, d0:d1], po)
                nc.gpsimd.dma_start(
                    out=out[sb * SB + mt * 128:sb * SB + (mt + 1) * 128, :],
                    in_=ob,
                )

    nc.finalize()
    return nc


def prepare_in_maps(x, c_fc, c_proj, gamma, mult_bias):
    bf16 = ml_dtypes.bfloat16
    g = (gamma.astype(np.float32) * np.float32(np.sqrt(D)))
    w1_all = (c_fc.astype(np.float32) * g[None, :, None]).astype(bf16)
    w2_all = (c_proj.astype(np.float32)
              * mult_bias.astype(np.float32)[None, :, None]).astype(bf16)
    xs = np.ascontiguousarray(np.transpose(x, (1, 0, 2, 3))).reshape(E, T, D)
    xs = xs.astype(bf16)
    xts = np.ascontiguousarray(np.transpose(xs, (0, 2, 1)))
    sel = np.zeros((S, SB), bf16)
    for s in range(S):
        sel[s, s * 128:(s + 1) * 128] = 1.0
    return [
        {"x": xs[e], "xT": xts[e], "w1": w1_all[e], "w2": w2_all[e], "sel": sel}
        for e in range(E)
    ]


def run(in_maps, trace: bool = False):
    nc = build_kernel()
    return run_bass_kernel_spmd(
        nc, in_maps, core_ids=list(range(E)), trace=trace,
    )


def kernel(x, c_fc, c_proj, gamma, mult_bias):
    in_maps = prepare_in_maps(x, c_fc, c_proj, gamma, mult_bias)
    res = run(in_maps)
    out = np.empty((E, B, CAP, D), np.float32)
    for e in range(E):
        out[e] = res.results[e]["out"].astype(np.float32).reshape(B, CAP, D)
    return np.ascontiguousarray(out.transpose(1, 0, 2, 3))


# revision 19
# speedup vs baseline: 1.1811x; 1.1811x over previous
"""Expert-parallel MoE GEGLU MLP (RMSNorm -> c_fc -> GEGLU -> c_proj) on 8
Trainium2 NeuronCores.

Sharding: expert-parallel. Core e computes the full MLP for expert e's tokens
(x[:, e] -> [8192, 768]); no collectives. gamma*sqrt(D) is folded into c_fc
and mult_bias into c_proj on the host, so the device kernel computes:

    h   = x / ||x||_2            (per token, fp32 accumulate)
    u   = h @ W1                 (bf16 x bf16 -> fp32 PSUM)
    g   = gelu(u_gate) * u_val   (exact erf gelu on ACT)
    out = g @ W2                 (bf16 x bf16 -> fp32 PSUM)

Layout: tokens stream in super-blocks of 1024. x is loaded twice: once
token-major (for the squared-sum only) and once d-major via the DMA xbar
transpose straight from DRAM. The per-token rsqrt scale is computed
token-major (cheap DVE Newton), moved to a row with one tiny PE transpose,
broadcast across partitions with K=8 bf16 hi/lo matmul pairs that accumulate
in PSUM (full fp32-grade precision at 1 cycle/row; the old fp32 selector
matmuls ran at 4 cycles/row with double LDWEIGHTS), and applied in place to
the transposed activations with DVE reads straight out of PSUM. GEMM1 runs
with hidden on PSUM partitions and 1024-token moving operands; GEMM2 uses the
GEGLU output chunks as the stationary operand so its PSUM output is already
token-major - no output transposes at all.

DMA queues: the latency-critical x stream (xb + xT) owns the sync queue; the
19MB weight bulk streams on the scalar queue behind only the first w1 column
pair; outputs drain on gpsimd. The first super-block's xb lands in two halves
so the ACT square chain starts ~4us earlier, and a short burst of dummy
matmuls warms the PE p-state while the first DMAs are in flight.
"""

from contextlib import ExitStack

import ml_dtypes
import numpy as np

import concourse.bass as bass
import concourse.mybir as mybir
import concourse.tile as tile
from concourse import bacc
from concourse.bass_utils import run_bass_kernel_spmd
from concourse.masks import make_identity

# Problem dims (fixed by the nn_MLP_90795608637901 spec).
B, E, CAP, D = 8, 8, 1024, 768
H = 2048
H2 = 2 * H
T = B * CAP          # tokens per expert (per core) = 8192
SB = 1024            # tokens per super-block
NSB = T // SB        # 8
S = SB // 128        # 8 partition sub-tiles per super-block
KC1 = D // 128       # 6 contraction chunks for GEMM1
MC = H // 128        # 16 value/gate chunk pairs
KC2 = H // 128       # 16 contraction chunks for GEMM2

BF = mybir.dt.bfloat16
F32 = mybir.dt.float32
I32 = mybir.dt.int32
ALU = mybir.AluOpType


def build_kernel(nsb: int = NSB) -> bass.Bass:
    nc = bacc.Bacc("TRN2", target_bir_lowering=False, debug=False)

    t = nsb * SB
    x = nc.declare_dram_parameter("x", [t, D], BF, isOutput=False)
    xT = nc.declare_dram_parameter("xT", [D, t], BF, isOutput=False)
    w1 = nc.declare_dram_parameter("w1", [D, H2], BF, isOutput=False)
    w2 = nc.declare_dram_parameter("w2", [H, D], BF, isOutput=False)
    sel = nc.declare_dram_parameter("sel", [S, SB], BF, isOutput=False)
    out = nc.declare_dram_parameter("out", [t, D], BF, isOutput=True)

    with tile.TileContext(nc) as tc, ExitStack() as ctx:
        weights = ctx.enter_context(tc.tile_pool(name="weights", bufs=1))
        io_in = ctx.enter_context(tc.tile_pool(name="io_in", bufs=3))
        work = ctx.enter_context(tc.tile_pool(name="work", bufs=2))
        gpool = ctx.enter_context(tc.tile_pool(name="gpool", bufs=1))
        small = ctx.enter_context(tc.tile_pool(name="small", bufs=2))
        agp = ctx.enter_context(tc.tile_pool(name="agp", bufs=3))
        obp = ctx.enter_context(tc.tile_pool(name="obp", bufs=3))
        psum_mm = ctx.enter_context(tc.tile_pool(name="psum_mm", bufs=5, space="PSUM"))
        psum_sc = ctx.enter_context(tc.tile_pool(name="psum_sc", bufs=1, space="PSUM"))
        psum_yt = ctx.enter_context(tc.tile_pool(name="psum_yt", bufs=1, space="PSUM"))

        # x DMAs for a super-block, all on the sync queue (the x stream is
        # latency-critical; the weight bulk rides the scalar queue). xb
        # issues one iteration ahead of xt: the scheduler interleaves block
        # sb+1's ACT squares among block sb's gelus, so a late xb stalls the
        # ACT FIFO and with it the PSUM-bank recycling the PE depends on.
        xb_tiles = {}
        xt_tiles = {}

        def issue_xb(sb):
            xb = io_in.tile([128, S, D], BF, name="xb", tag="xb")
            xv = x[sb * SB:(sb + 1) * SB].rearrange("(s p) d -> p s d", p=128)
            nc.sync.dma_start(out=xb, in_=xv)
            xb_tiles[sb] = xb

        def issue_xt(sb):
            xt = work.tile([128, KC1, SB], BF, name="xt", tag="xt")
            for k in range(KC1):
                nc.sync.dma_start(
                    out=xt[:, k, :],
                    in_=xT[k * 128:(k + 1) * 128, sb * SB:(sb + 1) * SB],
                )
            xt_tiles[sb] = xt

        # --- tiny constants (gpsimd/vector, ~0) ---
        ident = weights.tile([128, 128], F32)
        make_identity(nc, ident)
        # sel[s, s*128+q] = 1: selector for the partition-broadcast matmul
        # (bf16: 0/1 are exact).
        sels = weights.tile([S, SB], BF)
        bias0 = weights.tile([128, 1], F32)
        nc.vector.memset(bias0, 0.0)
        warm = weights.tile([128, 128], BF)
        nc.gpsimd.memset(warm, 0.0)

        # --- startup-ordered DMA head ---
        # sync queue: xb0 in two halves (ACT squares start after half 1),
        # then xt0. scalar queue: first w1 column pair, then the bulk.
        xb0 = io_in.tile([128, S, D], BF, name="xb", tag="xb")
        for h in range(2):
            sl = slice(h * (SB // 2), (h + 1) * (SB // 2))
            nc.sync.dma_start(
                out=xb0[:, h * (S // 2):(h + 1) * (S // 2), :],
                in_=x[sl].rearrange("(s p) d -> p s d", p=128),
            )
        xb_tiles[0] = xb0
        issue_xt(0)
        nc.sync.dma_start(out=sels, in_=sel[:, :])

        # Weight bulk: per-chunk [128 x 512] DMAs (1-2KB lines spread across
        # the 16 DMA engines). The first two w1 column pairs issue from the
        # gpsimd queue so they race sb0's x stream; the rest rides the sync
        # queue behind it. The ACT engine issues nothing.
        w1s = weights.tile([128, KC1, H2], BF)

        def w1_pair(nb, eng):
            for base in (0, H):
                c0, c1 = base + nb * 512, base + (nb + 1) * 512
                for k in range(KC1):
                    eng.dma_start(out=w1s[:, k, c0:c1],
                                  in_=w1[k * 128:(k + 1) * 128, c0:c1])

        w1_pair(0, nc.gpsimd)
        w1_pair(1, nc.gpsimd)

        # Warm the PE p-state while the first DMAs stream: ~24 dummy
        # 128-row matmuls on a zeroed tile (no data deps, scheduled at t~8us).
        pwarm = psum_yt.tile([128, 128], F32, name="pwarm", tag="yt", space="PSUM")
        for _ in range(24):
            nc.tensor.matmul(pwarm, lhsT=warm, rhs=warm, start=True, stop=True)

        w1_pair(2, nc.sync)
        w1_pair(3, nc.sync)
        if nsb > 1:
            issue_xb(1)
        w2s = weights.tile([128, KC2, D], BF)
        for k in range(KC2):
            nc.sync.dma_start(out=w2s[:, k, :], in_=w2[k * 128:(k + 1) * 128, :])

        normed = {}

        def norm_pipeline(sb):
            xb = xb_tiles.pop(sb)
            xt = xt_tiles[sb]
            # --- RMSNorm scale, token-major: ss on ACT, rsqrt on DVE ---
            # For the first super-block the chain is the startup critical
            # path, so half the squared-sums run on DVE concurrently.
            ssb = small.tile([128, S], F32, name="ssb")
            sq = small.tile([128, D], BF, name="sq")
            for s in range(S):
                nc.scalar.activation(
                    sq, xb[:, s], mybir.ActivationFunctionType.Square,
                    bias=bias0, accum_out=ssb[:, s:s + 1],
                )
            yb = small.tile([128, S], F32, name="yb")
            tb = small.tile([128, S], F32, name="tb")
            # rsqrt seed via the int bit trick: 0x5f3759df - (i >> 1)
            # (written as (i>>1 xor -1) + 0x5f3759df + 1), then 3 Newton steps.
            nc.vector.tensor_scalar(
                out=yb.bitcast(I32), in0=ssb.bitcast(I32),
                scalar1=1, scalar2=-1,
                op0=ALU.logical_shift_right, op1=ALU.bitwise_xor,
            )
            nc.vector.tensor_scalar(
                out=yb.bitcast(I32), in0=yb.bitcast(I32),
                scalar1=0x5F375A60, scalar2=None, op0=ALU.add,
            )
            # Two Newton steps suffice: seed err <= 3.4% -> 1.7e-3 -> 4.4e-6.
            for _ in range(2):
                nc.vector.tensor_mul(tb, yb, yb)
                nc.vector.tensor_mul(tb, tb, ssb)
                nc.vector.tensor_scalar(
                    out=tb, in0=tb, scalar1=-0.5, scalar2=1.5,
                    op0=ALU.mult, op1=ALU.add,
                )
                nc.vector.tensor_mul(yb, yb, tb)

            # --- broadcast scale across partitions: yb[p,s] -> psc[:,s*128+p]
            # One tiny fp32 transpose, then bf16 hi/lo selector matmul pairs
            # accumulating in PSUM: exact to ~16 mantissa bits, 1 cycle/row.
            yt = psum_yt.tile([S, 128], F32, name="yt", tag="yt", space="PSUM")
            nc.tensor.transpose(yt, yb, ident)
            yrow = small.tile([S, 128], F32, name="yrow")
            nc.vector.tensor_copy(yrow, yt)
            yhi = small.tile([S, 128], BF, name="yhi")
            nc.vector.tensor_copy(yhi, yrow)
            ylo = small.tile([S, 128], BF, name="ylo")
            nc.vector.tensor_sub(ylo, yrow, yhi)
            psc = psum_sc.tile([128, SB], F32, name="psc", tag="sc", space="PSUM")
            for s in range(S):
                cols = slice(s * 128, (s + 1) * 128)
                nc.tensor.matmul(psc[:, cols], lhsT=sels[:, cols], rhs=yhi,
                                 start=True, stop=False)
                nc.tensor.matmul(psc[:, cols], lhsT=sels[:, cols], rhs=ylo,
                                 start=False, stop=True)

            # --- normalize in place in the transposed domain (DVE reads the
            # scale straight out of PSUM). Half-column passes so GEMM1's
            # first 512-token chains start after 6 muls, not 12. ---
            for hc in range(2):
                cols = slice(hc * 512, (hc + 1) * 512)
                for k in range(KC1):
                    nc.vector.tensor_mul(xt[:, k, cols], xt[:, k, cols],
                                         psc[:, cols])
            normed[sb] = xt

        norm_pipeline(0)
        for sb in range(nsb):
            if sb + 1 < nsb:
                issue_xt(sb + 1)
            if sb + 2 < nsb:
                issue_xb(sb + 2)
            xt = normed.pop(sb)
            del xt_tiles[sb]

            # --- GEMM1 + GEGLU, one value/gate chunk pair at a time.
            # A matmul's fp32 PSUM output cannot cross a 2KB bank, so the
            # 1024-token super-block runs as two 512-column halves. ---
            gbuf = gpool.tile([128, KC2, SB], BF, name="gbuf")
            for m in range(MC):
                for h2 in range(2):
                    cols = slice(h2 * 512, (h2 + 1) * 512)
                    pv = psum_mm.tile([128, 512], F32, name="pv", tag="mm",
                                      space="PSUM")
                    pg = psum_mm.tile([128, 512], F32, name="pg", tag="mm",
                                      space="PSUM")
                    for k in range(KC1):
                        nc.tensor.matmul(
                            pv, lhsT=w1s[:, k, m * 128:(m + 1) * 128],
                            rhs=xt[:, k, cols],
                            start=(k == 0), stop=(k == KC1 - 1),
                        )
                    for k in range(KC1):
                        nc.tensor.matmul(
                            pg, lhsT=w1s[:, k, H + m * 128:H + (m + 1) * 128],
                            rhs=xt[:, k, cols],
                            start=(k == 0), stop=(k == KC1 - 1),
                        )
                    ag = agp.tile([128, 512], F32, name="ag")
                    nc.scalar.activation(
                        ag, pg, mybir.ActivationFunctionType.Gelu, bias=bias0,
                    )
                    nc.vector.tensor_mul(gbuf[:, m, cols], pv, ag)

            if sb + 1 < nsb:
                norm_pipeline(sb + 1)

            # --- GEMM2 with gbuf chunks stationary: PSUM comes out
            # token-major, so results DMA straight out after one copy.
            # d=768 output splits into 512+256 PSUM chains (bank rule). ---
            for mt in range(S):
                ob = obp.tile([128, D], BF, name="ob")
                for d0, d1 in ((0, 512), (512, 768)):
                    po = psum_mm.tile([128, d1 - d0], F32, name="po", tag="mm",
                                      space="PSUM")
                    for k2 in range(KC2):
                        nc.tensor.matmul(
                            po, lhsT=gbuf[:, k2, mt * 128:(mt + 1) * 128],
                            rhs=w2s[:, k2, d0:d1],
                            start=(k2 == 0), stop=(k2 == KC2 - 1),
                        )
                    nc.vector.tensor_copy(ob[:, d0:d1], po)
                nc.gpsimd.dma_start(
                    out=out[sb * SB + mt * 128:sb * SB + (mt + 1) * 128, :],
                    in_=ob,
                )

    nc.finalize()
    return nc


def prepare_in_maps(x, c_fc, c_proj, gamma, mult_bias):
    bf16 = ml_dtypes.bfloat16
    g = (gamma.astype(np.float32) * np.float32(np.sqrt(D)))
    w1_all = (c_fc.astype(np.float32) * g[None, :, None]).astype(bf16)
    w2_all = (c_proj.astype(np.float32)
              * mult_bias.astype(np.float32)[None, :, None]).astype(bf16)
    xs = np.ascontiguousarray(np.transpose(x, (1, 0, 2, 3))).reshape(E, T, D)
    xs = xs.astype(bf16)
    xts = np.ascontiguousarray(np.transpose(xs, (0, 2, 1)))
    sel = np.zeros((S, SB), bf16)
    for s in range(S):
        sel[s, s * 128:(s + 1) * 128] = 1.0
    return [
        {"x": xs[e], "xT": xts[e], "w1": w1_all[e], "w2": w2_all[e], "sel": sel}
        for e in range(E)
    ]


def run(in_maps, trace: bool = False):
    nc = build_kernel()
    return run_bass_kernel_spmd(
        nc, in_maps, core_ids=list(range(E)), trace=trace,
    )


def kernel(x, c_fc, c_proj, gamma, mult_bias):
    in_maps = prepare_in_maps(x, c_fc, c_proj, gamma, mult_bias)
    res = run(in_maps)
    out = np.empty((E, B, CAP, D), np.float32)
    for e in range(E):
        out[e] = res.results[e]["out"].astype(np.float32).reshape(B, CAP, D)
    return np.ascontiguousarray(out.transpose(1, 0, 2, 3))
